# revision 1
# baseline (speedup 1.0000x reference)
import math
import sys

sys.path.insert(0, "/opt/trn_rl_repo")

import numpy as np
import ml_dtypes

bf16np = ml_dtypes.bfloat16

# ---------------- problem constants (hardcoded; kernel.py must be self-contained) ----
B, T, S, D, H, L, DFF, IN, PERIOD = 16, 600, 600, 1024, 8, 8, 4096, 52, 25
HD = D // H          # 128
NC = 8               # cores
BC = B // NC         # 2 batches per core
DI = D // 128        # 8 i-tiles
DFI = DFF // 128     # 32
INV = 1.0 / math.sqrt(HD)
TK = 1 + T           # 601 keys (adapter + T)
# chunks of the 600-wide token dim (psum bank = 512 f32)
TCH = [(0, 512), (512, 88)]
KCH = [(0, 512), (512, 89)]          # 601-wide
KT = [(0, 128), (128, 128), (256, 128), (384, 128), (512, 89)]  # key tiles of 601

_cache = {}


def _build():
    """Build the per-core Bass graph (SPMD; same program all 8 cores)."""
    from concourse import bacc, mybir
    import concourse.bass as bass
    import concourse.bass_isa as bass_isa
    import concourse.tile as tile

    f32 = mybir.dt.float32
    bf = mybir.dt.bfloat16
    i32 = mybir.dt.int32
    AF = mybir.ActivationFunctionType
    OP = mybir.AluOpType
    RED = bass_isa.ReduceOp

    nc = bacc.Bacc("TRN2", target_bir_lowering=False, debug=False, num_devices=NC)

    def din(name, shape, dt=f32):
        return nc.dram_tensor(name, shape, dt, kind="ExternalInput").ap()

    # ---- DRAM inputs (host-prepped layouts) ----
    xT = din("xT", [BC, IN, T], bf)              # x transposed, bf16
    memT = din("memT", [BC, D, T], bf)           # memory transposed, bf16
    tsf = din("tsf", [1, BC])                    # timesteps as f32
    efm = din("efm", [128, DI])                  # e/(2pi) tiled per i-tile col
    phs = din("phs", [128, DI])                  # phase (0 / .25)
    peT = din("peT", [D, T])                     # pe.T + b_in  (f32)
    w_inT = din("w_inT", [IN, D], bf)
    te_w1T = din("te_w1T", [D, D], bf)
    te_w2T = din("te_w2T", [D, D], bf)
    te_b1t = din("te_b1t", [128, DI])
    te_b2t = din("te_b2t", [128, DI])
    sa_wqkvT = din("sa_wqkvT", [L, D, 3 * D], bf)
    sa_bqkvt = din("sa_bqkvt", [L, 128, 3 * DI])  # pre-tiled [128, 24]; q-part prescaled by INV
    sa_bvrow = din("sa_bvrow", [L, 1, D], bf)     # v-bias as row (for ones-MM trick)
    sa_woT = din("sa_woT", [L, D, D], bf)
    sa_bot = din("sa_bot", [L, 128, DI])
    ca_wqkvT = din("ca_wqkvT", [L, D, 3 * D], bf)
    ca_bqkvt = din("ca_bqkvt", [L, 128, 3 * DI])
    ca_woT = din("ca_woT", [L, D, D], bf)
    ca_bot = din("ca_bot", [L, 128, DI])
    ff_w1T = din("ff_w1T", [L, D, DFF], bf)
    ff_b1t = din("ff_b1t", [L, 128, DFI])
    ff_w2T = din("ff_w2T", [L, DFF, D], bf)
    ff_b2t = din("ff_b2t", [L, 128, DI])
    lngt = din("lngt", [L, 3, 128, DI])
    lnbt = din("lnbt", [L, 3, 128, DI])
    biasT = din("biasT", [H, TK, T])             # SA alibi bias transposed [h, k, q], f32
    w_outT = din("w_outT", [D, IN], bf)
    b_out = din("b_out", [IN, 1])
    out_d = nc.dram_tensor("out", [BC, IN, T], f32, kind="ExternalOutput").ap()

    with tile.TileContext(nc) as tc:
        res = tc.alloc_tile_pool(name="res", bufs=1)      # persistent
        w6 = tc.alloc_tile_pool(name="w6", bufs=32)       # bf16 [128,601] q/k/pt/attn ws
        fa = tc.alloc_tile_pool(name="fa", bufs=17)       # bf16 [128,601] ffa tiles
        hqp = tc.alloc_tile_pool(name="hqp", bufs=12)     # bf16 [128,601] LN targets
        mxp = tc.alloc_tile_pool(name="mxp", bufs=9)      # bf16 [128,601] mem staging
        vs = tc.alloc_tile_pool(name="vs", bufs=6)        # bf16 [128,1024] V tiles
        f6 = tc.alloc_tile_pool(name="f6", bufs=12)       # f32 [128,600] workspace
        wg = tc.alloc_tile_pool(name="wg", bufs=22)       # bf16 [128,512] weights
        bp = tc.alloc_tile_pool(name="bp", bufs=2)        # f32 [128,600] bias/pe stream
        sm = tc.alloc_tile_pool(name="sm", bufs=1)        # small persistents
        pk = tc.alloc_tile_pool(name="pk", bufs=6, space="PSUM")

        # persistent residual f32 and bf16 carrier (adapter col 0) - ONE batch at a time
        hf1 = [res.tile([128, T], f32, tag=f"hf_{i}", name=f"hf_{i}") for i in range(DI)]
        hx1 = [res.tile([128, TK], bf, tag=f"hx_{i}", name=f"hx_{i}") for i in range(DI)]
        hf = [hf1 for _ in range(BC)]
        hx = [hx1 for _ in range(BC)]

        ones_f = sm.tile([1, 128], f32, tag="ones_f", name="ones_f")
        nc.vector.memset(ones_f[:], 1.0)
        ones_b = sm.tile([1, 128], bf, tag="ones_b", name="ones_b")
        nc.vector.memset(ones_b[:], 1.0)
        eft = sm.tile([128, DI], f32, tag="eft", name="eft")
        nc.sync.dma_start(eft[:], efm[:])
        pht = sm.tile([128, DI], f32, tag="pht", name="pht")
        nc.sync.dma_start(pht[:], phs[:])
        tst = sm.tile([1, BC], f32, tag="tst", name="tst")
        eps_t = sm.tile([128, 1], f32, tag="eps_t", name="eps_t")
        nc.vector.memset(eps_t[:], 1e-5)
        nc.sync.dma_start(tst[:], tsf[:])
        adp = [sm.tile([128, BC], bf, tag=f"adp{i}", name=f"adp{i}") for i in range(DI)]  # adapter bf16

        # ---------- timestep embedding ----------
        ptb = pk.tile([128, BC], f32, tag="pk", name="ptb")
        nc.tensor.matmul(ptb[:], ones_f[:], tst[:], start=True, stop=True)  # t bcast f32
        temb = []
        for i in range(DI):
            y = sm.tile([128, BC], f32, tag=f"y{i}", name=f"y{i}")
            nc.vector.tensor_scalar_mul(y[:], ptb[:], eft[:, i : i + 1])
            nc.vector.tensor_scalar_add(y[:], y[:], pht[:, i : i + 1])
            yi = sm.tile([128, BC], i32, tag=f"yi{i}", name=f"yi{i}")
            nc.vector.tensor_copy(yi[:], y[:])
            yr = sm.tile([128, BC], f32, tag=f"yr{i}", name=f"yr{i}")
            nc.vector.tensor_copy(yr[:], yi[:])
            fr = sm.tile([128, BC], f32, tag=f"fr{i}", name=f"fr{i}")
            nc.vector.tensor_sub(fr[:], y[:], yr[:])
            tb = sm.tile([128, BC], bf, tag=f"tb{i}", name=f"tb{i}")
            nc.scalar.activation(tb[:], fr[:], AF.Sin, scale=2 * math.pi)
            temb.append(tb)

        def mlp1024(wT_d, bt_d, ins, act, outs_tag):
            """[D,D] proj on BC-wide f-major input tiles. Returns 8 bf16 [128,BC] tiles."""
            bt = sm.tile([128, DI], f32, tag=outs_tag + "_b", name=outs_tag + "_b")
            nc.sync.dma_start(bt[:], bt_d[:])
            outs = []
            for og in range(2):
                wts = []
                for i in range(DI):
                    w = wg.tile([128, 512], bf, tag="wg", name="wg")
                    nc.sync.dma_start(w[:], wT_d[i * 128 : (i + 1) * 128, og * 512 : (og + 1) * 512])
                    wts.append(w)
                for ot in range(4):
                    o = og * 4 + ot
                    p = pk.tile([128, BC], f32, tag="pk", name="pmlp")
                    for i in range(DI):
                        nc.tensor.matmul(p[:], wts[i][:, ot * 128 : (ot + 1) * 128], ins[i][:],
                                         start=(i == 0), stop=(i == DI - 1))
                    ob = sm.tile([128, BC], bf, tag=f"{outs_tag}{o}", name=f"{outs_tag}{o}")
                    nc.scalar.activation(ob[:], p[:], act, bias=bt[:, o : o + 1])
                    outs.append(ob)
            return outs

        z1 = mlp1024(te_w1T, te_b1t, temb, AF.Silu, "z1")
        z2 = mlp1024(te_w2T, te_b2t, z1, AF.Identity, "z2")
        for i in range(DI):
            nc.vector.tensor_copy(adp[i][:], z2[i][:])

        # ---------- helpers ----------
        def layernorm(b, g_ap, b_ap, tgt, tgt_off):
            """LN over features of X held in hf[b] (in place); bf16 copy to tgt[o][:, off:off+T]."""
            sacc = f6.tile([128, T], f32, tag="f6", name="f6")
            nc.vector.tensor_tensor(sacc[:], hf[b][0][:], hf[b][1][:], OP.add)
            for o in range(2, DI):
                nc.vector.tensor_tensor(sacc[:], sacc[:], hf[b][o][:], OP.add)
            qacc = f6.tile([128, T], f32, tag="f6", name="f6")
            tmp = f6.tile([128, T], f32, tag="f6", name="f6")
            nc.vector.tensor_tensor(qacc[:], hf[b][0][:], hf[b][0][:], OP.mult)
            for o in range(1, DI):
                nc.vector.tensor_tensor(tmp[:], hf[b][o][:], hf[b][o][:], OP.mult)
                nc.vector.tensor_tensor(qacc[:], qacc[:], tmp[:], OP.add)
            s1 = f6.tile([128, T], f32, tag="f6", name="f6")
            nc.gpsimd.partition_all_reduce(s1[:], sacc[:], channels=128, reduce_op=RED.add)
            s2 = f6.tile([128, T], f32, tag="f6", name="f6")
            nc.gpsimd.partition_all_reduce(s2[:], qacc[:], channels=128, reduce_op=RED.add)
            m = f6.tile([128, T], f32, tag="f6", name="f6")
            nc.vector.tensor_scalar_mul(m[:], s1[:], 1.0 / D)
            m2 = f6.tile([128, T], f32, tag="f6", name="f6")
            nc.vector.tensor_tensor(m2[:], m[:], m[:], OP.mult)
            var = f6.tile([128, T], f32, tag="f6", name="f6")
            nc.vector.scalar_tensor_tensor(var[:], s2[:], 1.0 / D, m2[:], OP.mult, OP.subtract)
            sd = f6.tile([128, T], f32, tag="f6", name="f6")
            nc.scalar.activation(sd[:], var[:], AF.Sqrt, bias=eps_t[:])
            r = f6.tile([128, T], f32, tag="f6", name="f6")
            nc.vector.reciprocal(r[:], sd[:])
            mr = f6.tile([128, T], f32, tag="f6", name="f6")
            nc.vector.tensor_tensor(mr[:], m[:], r[:], OP.mult)
            for o in range(DI):
                t1 = f6.tile([128, T], f32, tag="f6", name="f6")
                nc.vector.tensor_tensor(t1[:], hf[b][o][:], r[:], OP.mult)
                nc.vector.tensor_tensor(t1[:], t1[:], mr[:], OP.subtract)
                nc.scalar.activation(hf[b][o][:], t1[:], AF.Identity,
                                     bias=b_ap[:, o : o + 1], scale=g_ap[:, o : o + 1])
                nc.vector.tensor_copy(tgt[o][:, tgt_off : tgt_off + T], hf[b][o][:])

        def proj_res(b, wT_l, bot_ap, rhs_tiles, rhs_off):
            """out-proj [D,D] + bias + residual into hf[b] (X pre-LN)."""
            for og in range(2):
                wts = []
                for i in range(DI):
                    w = wg.tile([128, 512], bf, tag="wg", name="wg")
                    nc.sync.dma_start(w[:], wT_l[i * 128 : (i + 1) * 128, og * 512 : (og + 1) * 512])
                    wts.append(w)
                for ot in range(4):
                    o = og * 4 + ot
                    for c0, cw in TCH:
                        p = pk.tile([128, 512], f32, tag="pk", name="pk")
                        for i in range(DI):
                            nc.tensor.matmul(p[:, :cw], wts[i][:, ot * 128 : (ot + 1) * 128],
                                             rhs_tiles[i][:, rhs_off + c0 : rhs_off + c0 + cw],
                                             start=(i == 0), stop=(i == DI - 1))
                        nc.vector.scalar_tensor_tensor(hf[b][o][:, c0 : c0 + cw], p[:, :cw],
                                                       bot_ap[:, o : o + 1], hf[b][o][:, c0 : c0 + cw],
                                                       OP.add, OP.add)

        # per-layer bias tiles (re-DMAed each (b, l))
        sa_bq = sm.tile([128, 3 * DI], f32, tag="sa_bq", name="sa_bq")
        ca_bq = sm.tile([128, 3 * DI], f32, tag="ca_bq", name="ca_bq")
        sa_bo_t = sm.tile([128, DI], f32, tag="sa_bo_t", name="sa_bo_t")
        ca_bo_t = sm.tile([128, DI], f32, tag="ca_bo_t", name="ca_bo_t")
        f_b1 = sm.tile([128, DFI], f32, tag="f_b1", name="f_b1")
        f_b2 = sm.tile([128, DI], f32, tag="f_b2", name="f_b2")
        lng = [sm.tile([128, DI], f32, tag=f"lng{k}", name=f"lng{k}") for k in range(3)]
        lnb = [sm.tile([128, DI], f32, tag=f"lnb{k}", name=f"lnb{k}") for k in range(3)]
        bvr = sm.tile([1, D], bf, tag="bvr", name="bvr")
        bo_t = sm.tile([IN, 1], f32, tag="bo_t", name="bo_t")
        nc.sync.dma_start(bo_t[:], b_out[:])

        # ================= batch-serial main =================
        for b in range(BC):
            # ---------- input projection + pe ----------
            xb = sm.tile([IN, T], bf, tag="xb", name="xb")
            nc.sync.dma_start(xb[:], xT[b])
            for og in range(2):
                wts = []
                for ot in range(4):
                    o = og * 4 + ot
                    w = wg.tile([IN, 128], bf, tag="wgin", name="wgin")
                    nc.sync.dma_start(w[:], w_inT[:, o * 128 : (o + 1) * 128])
                    wts.append(w)
                for ot in range(4):
                    o = og * 4 + ot
                    pe_t = bp.tile([128, T], f32, tag="bp", name="bp")
                    nc.sync.dma_start(pe_t[:], peT[o * 128 : (o + 1) * 128, :])
                    for c0, cw in TCH:
                        p = pk.tile([128, 512], f32, tag="pk", name="pk")
                        nc.tensor.matmul(p[:, :cw], wts[ot][:], xb[:, c0 : c0 + cw],
                                         start=True, stop=True)
                        nc.vector.tensor_tensor(hf[b][o][:, c0 : c0 + cw], p[:, :cw],
                                                pe_t[:, c0 : c0 + cw], OP.add)
                    nc.vector.tensor_copy(hx[b][o][:, 1:], hf[b][o][:])
                    nc.vector.tensor_copy(hx[b][o][:, 0:1], adp[o][:, b : b + 1])

            for l in range(L):
                nc.sync.dma_start(sa_bq[:], sa_bqkvt[l])
                nc.sync.dma_start(ca_bq[:], ca_bqkvt[l])
                nc.sync.dma_start(sa_bo_t[:], sa_bot[l])
                nc.sync.dma_start(ca_bo_t[:], ca_bot[l])
                nc.sync.dma_start(f_b1[:], ff_b1t[l])
                nc.sync.dma_start(f_b2[:], ff_b2t[l])
                for k in range(3):
                    nc.sync.dma_start(lng[k][:], lngt[l, k])
                    nc.sync.dma_start(lnb[k][:], lnbt[l, k])
                nc.sync.dma_start(bvr[:], sa_bvrow[l])

                # ================= self-attention =================
                qsb = []
                ksb = []
                for og in range(4):
                    wts = []
                    for i in range(DI):
                        w = wg.tile([128, 512], bf, tag="wg", name="wg")
                        nc.sync.dma_start(w[:], sa_wqkvT[l, i * 128 : (i + 1) * 128,
                                                         og * 512 : (og + 1) * 512])
                        wts.append(w)
                    is_q = og < 2
                    for ot in range(4):
                        o = og * 4 + ot
                        dst = w6.tile([128, TK], bf, tag="w6", name="w6")
                        chunks = TCH if is_q else KCH
                        r0 = 1 if is_q else 0
                        for c0, cw in chunks:
                            p = pk.tile([128, 512], f32, tag="pk", name="pk")
                            for i in range(DI):
                                nc.tensor.matmul(p[:, :cw], wts[i][:, ot * 128 : (ot + 1) * 128],
                                                 hx[b][i][:, r0 + c0 : r0 + c0 + cw],
                                                 start=(i == 0), stop=(i == DI - 1))
                            nc.scalar.activation(dst[:, c0 : c0 + cw], p[:, :cw], AF.Identity,
                                                 bias=sa_bq[:, o : o + 1],
                                                 scale=INV if is_q else 1.0)
                        (qsb if is_q else ksb).append(dst)
                vsb = [vs.tile([128, 1024], bf, tag="vs", name="vs") for _ in range(5)]
                for og in range(4, 6):
                    wts = []
                    for i in range(DI):
                        w = wg.tile([128, 512], bf, tag="wg", name="wg")
                        nc.sync.dma_start(w[:], sa_wqkvT[l, i * 128 : (i + 1) * 128,
                                                         og * 512 : (og + 1) * 512])
                        wts.append(w)
                    vc0 = (og - 4) * 512
                    for kt_i, (k0, kw) in enumerate(KT):
                        p = pk.tile([128, 512], f32, tag="pk", name="pk")
                        for i in range(DI):
                            nc.tensor.matmul(p[:kw, :], hx[b][i][:, k0 : k0 + kw], wts[i][:],
                                             start=(i == 0), stop=False)
                        nc.tensor.matmul(p[:kw, :], ones_b[:, :kw],
                                         bvr[:, vc0 : vc0 + 512],
                                         start=False, stop=True)
                        nc.scalar.copy(vsb[kt_i][:kw, vc0 : vc0 + 512], p[:kw, :])
                attn = []
                for h in range(H):
                    pts = []
                    for kt_i, (k0, kw) in enumerate(KT):
                        bt = bp.tile([128, T], f32, tag="bp", name="bp")
                        nc.sync.dma_start(bt[:kw, :], biasT[h, k0 : k0 + kw, :])
                        sx = f6.tile([128, T], f32, tag="f6", name="f6")
                        for c0, cw in TCH:
                            p = pk.tile([128, 512], f32, tag="pk", name="pk")
                            nc.tensor.matmul(p[:kw, :cw], ksb[h][:, k0 : k0 + kw],
                                             qsb[h][:, c0 : c0 + cw], start=True, stop=True)
                            nc.vector.tensor_tensor(sx[:kw, c0 : c0 + cw], p[:kw, :cw],
                                                    bt[:kw, c0 : c0 + cw], OP.add)
                        pt = w6.tile([128, TK], bf, tag="w6", name="w6")
                        if kw < 128:
                            nc.vector.memset(pt[:, :T], 0.0)
                        nc.scalar.activation(pt[:kw, :T], sx[:kw, :], AF.Exp)
                        pts.append(pt)
                    dn = f6.tile([128, T], f32, tag="f6", name="f6")
                    nc.vector.tensor_tensor(dn[:], pts[0][:, :T], pts[1][:, :T], OP.add)
                    for kt_i in range(2, 5):
                        nc.vector.tensor_tensor(dn[:], dn[:], pts[kt_i][:, :T], OP.add)
                    dsum = f6.tile([128, T], f32, tag="f6", name="f6")
                    nc.gpsimd.partition_all_reduce(dsum[:], dn[:], channels=128, reduce_op=RED.add)
                    rb = f6.tile([128, T], f32, tag="f6", name="f6")
                    nc.vector.reciprocal(rb[:], dsum[:])
                    at = w6.tile([128, TK], bf, tag="w6", name="w6")
                    for c0, cw in TCH:
                        p = pk.tile([128, 512], f32, tag="pk", name="pk")
                        for kt_i, (k0, kw) in enumerate(KT):
                            nc.tensor.matmul(p[:, :cw], vsb[kt_i][:kw, h * 128 : (h + 1) * 128],
                                             pts[kt_i][:kw, c0 : c0 + cw],
                                             start=(kt_i == 0), stop=(kt_i == 4))
                        nc.vector.tensor_tensor(at[:, c0 : c0 + cw], p[:, :cw],
                                                rb[:, c0 : c0 + cw], OP.mult)
                    attn.append(at)
                proj_res(b, sa_woT[l], sa_bo_t[:], attn, 0)
                hq = [hqp.tile([128, TK], bf, tag="hqp", name="hqp") for _ in range(DI)]
                layernorm(b, lng[0][:], lnb[0][:], hq, 0)

                # ================= cross-attention =================
                mx = []
                for i in range(DI):
                    m_ = mxp.tile([128, TK], bf, tag="mxp", name="mxp")
                    nc.sync.dma_start(m_[:, 1:], memT[b, i * 128 : (i + 1) * 128, :])
                    nc.vector.tensor_copy(m_[:, 0:1], adp[i][:, b : b + 1])
                    mx.append(m_)
                ca_attn = []
                for hg in range(2):
                    wq_b = []
                    wk_b = []
                    for i in range(DI):
                        wq_ = wg.tile([128, 512], bf, tag="wg", name="wg")
                        nc.sync.dma_start(wq_[:], ca_wqkvT[l, i * 128 : (i + 1) * 128,
                                                           hg * 512 : hg * 512 + 512])
                        wq_b.append(wq_)
                        wk_ = wg.tile([128, 512], bf, tag="wg", name="wg")
                        nc.sync.dma_start(wk_[:], ca_wqkvT[l, i * 128 : (i + 1) * 128,
                                                           D + hg * 512 : D + hg * 512 + 512])
                        wk_b.append(wk_)
                    qs = []
                    ks = []
                    for hh in range(4):
                        h = hg * 4 + hh
                        osl = slice(hh * 128, (hh + 1) * 128)
                        qh = w6.tile([128, TK], bf, tag="w6", name="w6")
                        for c0, cw in TCH:
                            p = pk.tile([128, 512], f32, tag="pk", name="pk")
                            for i in range(DI):
                                nc.tensor.matmul(p[:, :cw], wq_b[i][:, osl],
                                                 hq[i][:, c0 : c0 + cw],
                                                 start=(i == 0), stop=(i == DI - 1))
                            nc.scalar.activation(qh[:, c0 : c0 + cw], p[:, :cw], AF.Identity,
                                                 bias=ca_bq[:, h : h + 1], scale=INV)
                        qs.append(qh)
                        kh = w6.tile([128, TK], bf, tag="w6", name="w6")
                        for c0, cw in KCH:
                            p = pk.tile([128, 512], f32, tag="pk", name="pk")
                            for i in range(DI):
                                nc.tensor.matmul(p[:, :cw], wk_b[i][:, osl],
                                                 mx[i][:, c0 : c0 + cw],
                                                 start=(i == 0), stop=(i == DI - 1))
                            nc.scalar.activation(kh[:, c0 : c0 + cw], p[:, :cw], AF.Identity,
                                                 bias=ca_bq[:, DI + h : DI + h + 1])
                        ks.append(kh)
                    wv_b = []
                    for i in range(DI):
                        wv_ = wg.tile([128, 512], bf, tag="wg", name="wg")
                        nc.sync.dma_start(wv_[:], ca_wqkvT[l, i * 128 : (i + 1) * 128,
                                                           2 * D + hg * 512 : 2 * D + hg * 512 + 512])
                        wv_b.append(wv_)
                    for hh in range(4):
                        h = hg * 4 + hh
                        osl = slice(hh * 128, (hh + 1) * 128)
                        qh = qs[hh]
                        kh = ks[hh]
                        vh = w6.tile([128, TK], bf, tag="w6", name="w6")
                        for c0, cw in KCH:
                            p = pk.tile([128, 512], f32, tag="pk", name="pk")
                            for i in range(DI):
                                nc.tensor.matmul(p[:, :cw], wv_b[i][:, osl],
                                                 mx[i][:, c0 : c0 + cw],
                                                 start=(i == 0), stop=(i == DI - 1))
                            nc.scalar.activation(vh[:, c0 : c0 + cw], p[:, :cw], AF.Identity,
                                                 bias=ca_bq[:, 2 * DI + h : 2 * DI + h + 1])
                        ka_f = f6.tile([128, 1], f32, tag="ka_f", name="ka_f")
                        nc.vector.tensor_copy(ka_f[:], kh[:, 0:1])
                        va_f = f6.tile([128, 1], f32, tag="va_f", name="va_f")
                        nc.vector.tensor_copy(va_f[:], vh[:, 0:1])
                        qk = f6.tile([128, T], f32, tag="f6", name="f6")
                        nc.vector.tensor_tensor(qk[:], qh[:, :T], kh[:, 1:], OP.mult)
                        smb = f6.tile([128, T], f32, tag="f6", name="f6")
                        nc.gpsimd.partition_all_reduce(smb[:], qk[:], channels=128, reduce_op=RED.add)
                        nc.vector.tensor_scalar_mul(qk[:], qh[:, :T], ka_f[:])
                        sab = f6.tile([128, T], f32, tag="f6", name="f6")
                        nc.gpsimd.partition_all_reduce(sab[:], qk[:], channels=128, reduce_op=RED.add)
                        dd = f6.tile([128, T], f32, tag="f6", name="f6")
                        nc.vector.tensor_tensor(dd[:], smb[:], sab[:], OP.subtract)
                        wm = f6.tile([128, T], f32, tag="f6", name="f6")
                        nc.scalar.activation(wm[:], dd[:], AF.Sigmoid)
                        at = w6.tile([128, TK], bf, tag="w6", name="w6")
                        t1 = f6.tile([128, T], f32, tag="f6", name="f6")
                        nc.vector.tensor_scalar_sub(t1[:], vh[:, 1:], va_f[:])
                        nc.vector.tensor_tensor(t1[:], t1[:], wm[:], OP.mult)
                        nc.vector.tensor_scalar_add(at[:, :T], t1[:], va_f[:])
                        ca_attn.append(at)
                proj_res(b, ca_woT[l], ca_bo_t[:], ca_attn, 0)
                hq = [hqp.tile([128, TK], bf, tag="hqp", name="hqp") for _ in range(DI)]
                layernorm(b, lng[1][:], lnb[1][:], hq, 0)

                # ================= FFN =================
                xacc = [f6.tile([128, T], f32, tag="f6", name="f6") for _ in range(DI)]
                for half in range(2):
                    ffa = []
                    for og in range(4):
                        go = half * 4 + og
                        wts = []
                        for i in range(DI):
                            w = wg.tile([128, 512], bf, tag="wg", name="wg")
                            nc.sync.dma_start(w[:], ff_w1T[l, i * 128 : (i + 1) * 128,
                                                          go * 512 : (go + 1) * 512])
                            wts.append(w)
                        for ot in range(4):
                            o = go * 4 + ot
                            dst = fa.tile([128, TK], bf, tag="fa", name="fa")
                            for c0, cw in TCH:
                                p = pk.tile([128, 512], f32, tag="pk", name="pk")
                                for i in range(DI):
                                    nc.tensor.matmul(p[:, :cw], wts[i][:, ot * 128 : (ot + 1) * 128],
                                                     hq[i][:, c0 : c0 + cw],
                                                     start=(i == 0), stop=(i == DI - 1))
                                nc.scalar.activation(dst[:, c0 : c0 + cw], p[:, :cw], AF.Relu,
                                                     bias=f_b1[:, o : o + 1])
                            ffa.append(dst)
                    for og2 in range(4):
                        w2t = []
                        for ii in range(16):
                            i_t = half * 16 + ii
                            w = wg.tile([128, 512], bf, tag="wg", name="wg")
                            nc.sync.dma_start(w[:, :256], ff_w2T[l, i_t * 128 : (i_t + 1) * 128,
                                                                 og2 * 256 : (og2 + 1) * 256])
                            w2t.append(w)
                        for o2t in range(2):
                            o = og2 * 2 + o2t
                            for c0, cw in TCH:
                                p = pk.tile([128, 512], f32, tag="pk", name="pk")
                                for ii in range(16):
                                    nc.tensor.matmul(p[:, :cw],
                                                     w2t[ii][:, o2t * 128 : o2t * 128 + 128],
                                                     ffa[ii][:, c0 : c0 + cw],
                                                     start=(ii == 0), stop=(ii == 15))
                                if half == 0:
                                    nc.vector.scalar_tensor_tensor(
                                        xacc[o][:, c0 : c0 + cw], p[:, :cw],
                                        f_b2[:, o : o + 1], hf[b][o][:, c0 : c0 + cw],
                                        OP.add, OP.add)
                                else:
                                    nc.vector.tensor_tensor(hf[b][o][:, c0 : c0 + cw],
                                                            p[:, :cw], xacc[o][:, c0 : c0 + cw],
                                                            OP.add)
                layernorm(b, lng[2][:], lnb[2][:], hx[b], 1)
                for i in range(DI):
                    nc.vector.tensor_copy(hx[b][i][:, 0:1], adp[i][:, b : b + 1])

            # ---------- output projection ----------
            wo_t = []
            for i in range(DI):
                w = wg.tile([128, IN], bf, tag="wgout", name="wgout")
                nc.sync.dma_start(w[:], w_outT[i * 128 : (i + 1) * 128, :])
                wo_t.append(w)
            ot_ = sm.tile([IN, T], f32, tag=f"osb{b}", name=f"osb{b}")
            for c0, cw in TCH:
                p = pk.tile([128, 512], f32, tag="pk", name="pk")
                for i in range(DI):
                    nc.tensor.matmul(p[:IN, :cw], wo_t[i][:], hx[b][i][:, 1 + c0 : 1 + c0 + cw],
                                     start=(i == 0), stop=(i == DI - 1))
                nc.scalar.activation(ot_[:, c0 : c0 + cw], p[:IN, :cw], AF.Identity, bias=bo_t[:])
            nc.sync.dma_start(out_d[b], ot_[:])

        for _pool in (pk, sm, bp, wg, f6, vs, mxp, hqp, fa, w6, res):
            _pool.release()

    nc.compile()
    return nc


def _prep_host(inputs):
    """Build the 8 per-core input maps from full inputs."""
    f32 = np.float32

    def b16(a):
        return np.ascontiguousarray(np.asarray(a, f32)).astype(bf16np)

    def tiled(vec, n):          # [n*128] -> [128, n] (col j = tile j)
        return np.ascontiguousarray(np.asarray(vec, f32).reshape(n, 128).T)

    x = np.asarray(inputs["x"], f32)
    memory = np.asarray(inputs["memory"], f32)
    ts = np.asarray(inputs["timesteps"])
    pe = np.asarray(inputs["pe"], f32)
    alibi = np.asarray(inputs["alibi"], f32)

    half = D // 2
    expo = np.exp(-math.log(10000.0) * np.arange(half, dtype=f32) / (half - 1.0))
    efm = np.concatenate([expo, expo]) / (2 * np.pi)
    phs = np.concatenate([np.zeros(half, f32), np.full(half, 0.25, f32)])

    biasT = np.concatenate([np.zeros((H, T, 1), f32), alibi], axis=-1).transpose(0, 2, 1)
    biasT = np.ascontiguousarray(biasT)

    qkv_bias = {}
    for nm in ("sa", "ca"):
        bq = np.asarray(inputs[f"{nm}_bqkv"], f32).copy()      # [L, 3D]
        bq[:, :D] *= INV                                       # pre-scale q bias
        qkv_bias[nm] = np.stack([np.stack([tiled(bq[l, k * 128 : (k + 1) * 128 * 0 + (k + 1) * 128], 1)[:, 0]
                                           for k in range(3 * DI)], axis=1)
                                 for l in range(L)])           # [L,128,24]

    common = {
        "tsf": None, "xT": None, "memT": None,
        "efm": tiled(efm, DI), "phs": tiled(phs, DI),
        "peT": np.ascontiguousarray(pe.T + np.asarray(inputs["b_in"], f32)[:, None]),
        "w_inT": b16(np.asarray(inputs["W_in"], f32).T),
        "te_w1T": b16(np.asarray(inputs["te_W1"], f32).T),
        "te_w2T": b16(np.asarray(inputs["te_W2"], f32).T),
        "te_b1t": tiled(inputs["te_b1"], DI),
        "te_b2t": tiled(inputs["te_b2"], DI),
        "sa_wqkvT": b16(np.asarray(inputs["sa_Wqkv"], f32).transpose(0, 2, 1)),
        "sa_bqkvt": qkv_bias["sa"],
        "sa_bvrow": b16(np.asarray(inputs["sa_bqkv"], f32)[:, 2 * D :][:, None, :]),
        "sa_woT": b16(np.asarray(inputs["sa_Wo"], f32).transpose(0, 2, 1)),
        "sa_bot": np.stack([tiled(np.asarray(inputs["sa_bo"], f32)[l], DI) for l in range(L)]),
        "ca_wqkvT": b16(np.asarray(inputs["ca_Wqkv"], f32).transpose(0, 2, 1)),
        "ca_bqkvt": qkv_bias["ca"],
        "ca_woT": b16(np.asarray(inputs["ca_Wo"], f32).transpose(0, 2, 1)),
        "ca_bot": np.stack([tiled(np.asarray(inputs["ca_bo"], f32)[l], DI) for l in range(L)]),
        "ff_w1T": b16(np.asarray(inputs["ff_W1"], f32).transpose(0, 2, 1)),
        "ff_b1t": np.stack([tiled(np.asarray(inputs["ff_b1"], f32)[l], DFI) for l in range(L)]),
        "ff_w2T": b16(np.asarray(inputs["ff_W2"], f32).transpose(0, 2, 1)),
        "ff_b2t": np.stack([tiled(np.asarray(inputs["ff_b2"], f32)[l], DI) for l in range(L)]),
        "lngt": np.stack([np.stack([tiled(np.asarray(inputs[f"ln{k+1}_g"], f32)[l], DI)
                                    for k in range(3)]) for l in range(L)]),
        "lnbt": np.stack([np.stack([tiled(np.asarray(inputs[f"ln{k+1}_b"], f32)[l], DI)
                                    for k in range(3)]) for l in range(L)]),
        "biasT": biasT,
        "w_outT": b16(np.asarray(inputs["W_out"], f32).T),
        "b_out": np.asarray(inputs["b_out"], f32)[:, None],
    }

    in_maps = []
    for c in range(NC):
        b0 = c * BC
        m = dict(common)
        m["xT"] = b16(x[b0 : b0 + BC].transpose(0, 2, 1))
        m["memT"] = b16(memory[b0 : b0 + BC].transpose(0, 2, 1))
        m["tsf"] = np.asarray(ts[b0 : b0 + BC], f32)[None, :]
        in_maps.append(m)
    return in_maps


def kernel(**inputs):
    from concourse.bass_utils import run_bass_kernel_spmd

    if "nc" not in _cache:
        _cache["nc"] = _build()
    nc = _cache["nc"]
    in_maps = _prep_host(inputs)
    res = run_bass_kernel_spmd(nc, in_maps, core_ids=list(range(NC)))
    out = np.empty((B, T, IN), np.float32)
    for c in range(NC):
        out[c * BC : (c + 1) * BC] = res.results[c]["out"].transpose(0, 2, 1)
    return out



# revision 2
# speedup vs baseline: 1.2057x; 1.2057x over previous
import math
import sys

sys.path.insert(0, "/opt/trn_rl_repo")

import numpy as np
import ml_dtypes

bf16np = ml_dtypes.bfloat16

# ---------------- problem constants ----------------
B, T, S, D, H, L, DFF, IN, PERIOD = 16, 600, 600, 1024, 8, 8, 4096, 52, 25
HD = D // H          # 128
NC = 8               # cores
BC = B // NC         # 2 batches per core
DI = D // 128        # 8 feature tiles
DFI = DFF // 128     # 32
INV = 1.0 / math.sqrt(HD)
W = 2 * (T + 1)      # 1202 merged cols: [adp0 | 600 b0 | adp1 | 600 b1]
A = (0, T + 1)       # adapter col per batch
CH3 = [(0, 401), (401, 401), (802, 400)]                 # merged col chunks
BQCH = [(0, 0, 301), (0, 301, 300), (1, 601, 301), (1, 902, 300)]  # (batch, c0, cw)
KT = [(0, 128), (128, 128), (256, 128), (384, 128), (512, 89)]     # per-batch key tiles

_cache = {}


def _build():
    from concourse import bacc, mybir
    import concourse.bass as bass
    import concourse.tile as tile

    f32 = mybir.dt.float32
    bf = mybir.dt.bfloat16
    i32 = mybir.dt.int32
    AF = mybir.ActivationFunctionType
    OP = mybir.AluOpType

    nc = bacc.Bacc("TRN2", target_bir_lowering=False, debug=False, num_devices=NC)

    def din(name, shape, dt=f32):
        return nc.dram_tensor(name, shape, dt, kind="ExternalInput").ap()

    # ---- DRAM inputs ----
    xP = din("xP", [IN, W], bf)                  # x merged-transposed
    memP = din("memP", [D, W], bf)               # memory merged-transposed
    tsf = din("tsf", [1, BC])
    efm = din("efm", [128, DI])
    phs = din("phs", [128, DI])
    pePB = din("pePB", [D, W], bf)               # pe.T + b_in, duplicated per batch
    w_inT = din("w_inT", [IN, D], bf)
    te_w1T = din("te_w1T", [D, D], bf)
    te_w2T = din("te_w2T", [D, D], bf)
    te_b1t = din("te_b1t", [128, DI])
    te_b2t = din("te_b2t", [128, DI])
    sa_wqkvT = din("sa_wqkvT", [L, D, 3 * D], bf)
    sa_bqkvt = din("sa_bqkvt", [L, 128, 3 * DI])  # q-part prescaled by INV
    sa_bvrow = din("sa_bvrow", [L, 1, D], bf)
    sa_woT = din("sa_woT", [L, D, D], bf)
    sa_bot = din("sa_bot", [L, 128, DI])
    ca_wqkvT = din("ca_wqkvT", [L, D, 3 * D], bf)
    ca_bqkvt = din("ca_bqkvt", [L, 128, 3 * DI])
    ca_woT = din("ca_woT", [L, D, D], bf)
    ca_bot = din("ca_bot", [L, 128, DI])
    ff_w1T = din("ff_w1T", [L, D, DFF], bf)
    ff_b1t = din("ff_b1t", [L, 128, DFI])
    ff_w2T = din("ff_w2T", [L, DFF, D], bf)
    ff_b2t = din("ff_b2t", [L, 128, DI])
    lngt = din("lngt", [L, 3, 128, DI])
    lnbt = din("lnbt", [L, 3, 128, DI])
    EBD = din("EBD", [H, T + 1, W], bf)          # exp(alibi), q-cols duplicated
    w_outT = din("w_outT", [D, IN], bf)
    b_out = din("b_out", [IN, 1])
    out_d = nc.dram_tensor("out", [IN, W], bf, kind="ExternalOutput").ap()

    with tile.TileContext(nc) as tc:
        res = tc.alloc_tile_pool(name="res", bufs=1)      # persistent state
        sm = tc.alloc_tile_pool(name="sm", bufs=1)        # small persistents
        bb = tc.alloc_tile_pool(name="bb", bufs=8)        # big bf16 tiles
        wg = tc.alloc_tile_pool(name="wg", bufs=24)       # weights [128,1024] bf16
        rw = tc.alloc_tile_pool(name="rw", bufs=4)        # row workspaces
        pk = tc.alloc_tile_pool(name="pk", bufs=3, space="PSUM")
        pd = tc.alloc_tile_pool(name="pd", bufs=2, space="PSUM")
        pb = tc.alloc_tile_pool(name="pb", bufs=2, space="PSUM")

        # persistent merged state: 8 tiles [128, W] bf16
        hs = [res.tile([128, W], bf, tag=f"hs{i}", name=f"hs{i}") for i in range(DI)]

        ones_b = sm.tile([1, 128], bf, tag="ones_b", name="ones_b")
        nc.vector.memset(ones_b[:], 1.0)
        ones_f = sm.tile([1, 128], f32, tag="ones_f", name="ones_f")
        nc.vector.memset(ones_f[:], 1.0)
        ones_col = sm.tile([128, 1], bf, tag="ones_col", name="ones_col")
        nc.vector.memset(ones_col[:], 1.0)
        eps_row = sm.tile([1, 1], f32, tag="eps_row", name="eps_row")
        nc.vector.memset(eps_row[:], 1e-5)
        eft = sm.tile([128, DI], f32, tag="eft", name="eft")
        nc.sync.dma_start(eft[:], efm[:])
        pht = sm.tile([128, DI], f32, tag="pht", name="pht")
        nc.sync.dma_start(pht[:], phs[:])
        tst = sm.tile([1, BC], f32, tag="tst", name="tst")
        nc.sync.dma_start(tst[:], tsf[:])
        adp = [sm.tile([128, BC], bf, tag=f"adp{i}", name=f"adp{i}") for i in range(DI)]

        # ---------- timestep embedding (as baseline) ----------
        ptb = pk.tile([128, BC], f32, tag="pk", name="ptb")
        nc.tensor.matmul(ptb[:], ones_f[:], tst[:], start=True, stop=True)
        temb = []
        for i in range(DI):
            y = sm.tile([128, BC], f32, tag=f"y{i}", name=f"y{i}")
            nc.vector.tensor_scalar_mul(y[:], ptb[:], eft[:, i : i + 1])
            nc.vector.tensor_scalar_add(y[:], y[:], pht[:, i : i + 1])
            yi = sm.tile([128, BC], i32, tag=f"yi{i}", name=f"yi{i}")
            nc.vector.tensor_copy(yi[:], y[:])
            yr = sm.tile([128, BC], f32, tag=f"yr{i}", name=f"yr{i}")
            nc.vector.tensor_copy(yr[:], yi[:])
            fr = sm.tile([128, BC], f32, tag=f"fr{i}", name=f"fr{i}")
            nc.vector.tensor_sub(fr[:], y[:], yr[:])
            tb = sm.tile([128, BC], bf, tag=f"tb{i}", name=f"tb{i}")
            nc.scalar.activation(tb[:], fr[:], AF.Sin, scale=2 * math.pi)
            temb.append(tb)

        def mlp1024(wT_d, bt_d, ins, act, outs_tag):
            bt = sm.tile([128, DI], f32, tag=outs_tag + "_b", name=outs_tag + "_b")
            nc.sync.dma_start(bt[:], bt_d[:])
            outs = []
            for og in range(2):
                wts = []
                for i in range(DI):
                    w = wg.tile([128, 1024], bf, tag="wg", name="wg")
                    nc.sync.dma_start(w[:, :512],
                                      wT_d[i * 128 : (i + 1) * 128, og * 512 : (og + 1) * 512])
                    wts.append(w)
                for ot in range(4):
                    o = og * 4 + ot
                    p = pk.tile([128, BC], f32, tag="pk", name="pmlp")
                    for i in range(DI):
                        nc.tensor.matmul(p[:], wts[i][:, ot * 128 : (ot + 1) * 128], ins[i][:],
                                         start=(i == 0), stop=(i == DI - 1))
                    ob = sm.tile([128, BC], bf, tag=f"{outs_tag}{o}", name=f"{outs_tag}{o}")
                    nc.scalar.activation(ob[:], p[:], act, bias=bt[:, o : o + 1])
                    outs.append(ob)
            return outs

        z1 = mlp1024(te_w1T, te_b1t, temb, AF.Silu, "z1")
        z2 = mlp1024(te_w2T, te_b2t, z1, AF.Identity, "z2")
        for i in range(DI):
            nc.vector.tensor_copy(adp[i][:], z2[i][:])

        def refresh_adapters():
            for i in range(DI):
                nc.vector.tensor_copy(hs[i][:, 0:1], adp[i][:, 0:1])
                nc.vector.tensor_copy(hs[i][:, A[1] : A[1] + 1], adp[i][:, 1:2])

        # ---------- input projection + pe ----------
        xb = bb.tile([IN, W], bf, tag="eb", bufs=4, name="xb")
        nc.sync.dma_start(xb[:], xP[:])
        w_in_t = []
        for o in range(DI):
            wt = sm.tile([IN, 128], bf, tag=f"win{o}", name=f"win{o}")
            nc.sync.dma_start(wt[:], w_inT[:, o * 128 : (o + 1) * 128])
            w_in_t.append(wt)
        for o in range(DI):
            peb = bb.tile([128, W], bf, tag="eb", bufs=4, name="peb")
            nc.sync.dma_start(peb[:], pePB[o * 128 : (o + 1) * 128, :])
            for c0, cw in CH3:
                p = pk.tile([128, 512], f32, tag="pk", name="pk")
                nc.tensor.matmul(p[:, :cw], w_in_t[o][:], xb[:, c0 : c0 + cw],
                                 start=True, stop=True)
                nc.vector.tensor_tensor(hs[o][:, c0 : c0 + cw], p[:, :cw],
                                        peb[:, c0 : c0 + cw], OP.add)
        refresh_adapters()

        # w_out resident
        w_out_t = []
        for i in range(DI):
            wt = sm.tile([128, IN], bf, tag=f"wout{i}", name=f"wout{i}")
            nc.sync.dma_start(wt[:], w_outT[i * 128 : (i + 1) * 128, :])
            w_out_t.append(wt)
        bo_t = sm.tile([IN, 1], f32, tag="bo_t", name="bo_t")
        nc.sync.dma_start(bo_t[:], b_out[:])

        # ---------- helpers ----------
        def load_w(src, n=8, cols=1024):
            ws = []
            for i in range(n):
                w = wg.tile([128, 1024], bf, tag="wg", name="wg")
                nc.sync.dma_start(w[:, :cols], src(i))
                ws.append(w)
            return ws

        def out_proj_res(wo_d, bo_ap, at_tiles):
            """attention out-projection [D,D] + bias + residual into hs."""
            wo = load_w(lambda i: wo_d[i * 128 : (i + 1) * 128, :1024])
            for c0, cw in CH3:
                for o in range(DI):
                    p = pk.tile([128, 512], f32, tag="pk", name="pk")
                    for i in range(DI):
                        nc.tensor.matmul(p[:, :cw], wo[i][:, o * 128 : (o + 1) * 128],
                                         at_tiles[i][:, c0 : c0 + cw],
                                         start=(i == 0), stop=(i == DI - 1))
                    nc.vector.scalar_tensor_tensor(hs[o][:, c0 : c0 + cw], p[:, :cw],
                                                   bo_ap[:, o : o + 1],
                                                   hs[o][:, c0 : c0 + cw],
                                                   OP.add, OP.add)

        def layer_norm(g_t, b_t):
            """in-place LN of hs over the feature dim (1024 = 8 tiles)."""
            s1l, s2l = [], []
            for c0, cw in CH3:
                s1 = pd.tile([1, 512], f32, tag="pd", name="s1")
                for i in range(DI):
                    nc.tensor.matmul(s1[:, :cw], ones_col[:], hs[i][:, c0 : c0 + cw],
                                     start=(i == 0), stop=(i == DI - 1))
                s2 = pd.tile([1, 512], f32, tag="pd", name="s2")
                for i in range(DI):
                    sq = bb.tile([128, 401], bf, tag="sq", bufs=2, name="sq")
                    nc.scalar.activation(sq[:, :cw], hs[i][:, c0 : c0 + cw], AF.Square)
                    nc.tensor.matmul(s2[:, :cw], ones_col[:], sq[:, :cw],
                                     start=(i == 0), stop=(i == DI - 1))
                # rows: m = s1/D; var = s2/D - m^2; r = 1/sqrt(var+eps); mr = m*r
                mrow = rw.tile([1, 401], f32, tag="mrow", bufs=1, name="mrow")
                nc.scalar.mul(mrow[:, :cw], s1[:, :cw], 1.0 / D)
                m2 = rw.tile([1, 401], f32, tag="m2", bufs=1, name="m2")
                nc.vector.tensor_tensor(m2[:, :cw], mrow[:, :cw], mrow[:, :cw], OP.mult)
                nc.vector.scalar_tensor_tensor(m2[:, :cw], s2[:, :cw], 1.0 / D,
                                               m2[:, :cw], OP.mult, OP.subtract)
                nc.scalar.activation(m2[:, :cw], m2[:, :cw], AF.Sqrt, bias=eps_row[:])
                rb_f = rw.tile([1, 401], f32, tag="rb_f", bufs=3, name="rb_f")
                nc.vector.reciprocal_approx_fast(rb_f[:, :cw], m2[:, :cw])
                mr_r = rw.tile([1, 401], f32, tag="mr_r", bufs=3, name="mr_r")
                nc.vector.tensor_tensor(mr_r[:, :cw], mrow[:, :cw], rb_f[:, :cw], OP.mult)
                s1l.append(rb_f)
                s2l.append(mr_r)
            for ci, (c0, cw) in enumerate(CH3):
                RBp = pb.tile([128, 512], f32, tag="pb", name="RBp")
                nc.tensor.matmul(RBp[:, :cw], ones_f[:], s1l[ci][:, :cw],
                                 start=True, stop=True)
                MRp = pb.tile([128, 512], f32, tag="pb", name="MRp")
                nc.tensor.matmul(MRp[:, :cw], ones_f[:], s2l[ci][:, :cw],
                                 start=True, stop=True)
                for i in range(DI):
                    t1 = bb.tile([128, 401], bf, tag="t1", bufs=2, name="t1")
                    nc.vector.tensor_tensor(t1[:, :cw], hs[i][:, c0 : c0 + cw],
                                            RBp[:, :cw], OP.mult)
                    nc.vector.tensor_tensor(t1[:, :cw], t1[:, :cw], MRp[:, :cw],
                                            OP.subtract)
                    nc.scalar.activation(hs[i][:, c0 : c0 + cw], t1[:, :cw], AF.Identity,
                                         bias=b_t[:, i : i + 1], scale=g_t[:, i : i + 1])

        # ================= layers =================
        for l in range(L):
            sa_bq = sm.tile([128, 3 * DI], f32, tag="sa_bq", bufs=2, name="sa_bq")
            nc.sync.dma_start(sa_bq[:], sa_bqkvt[l])
            ca_bq = sm.tile([128, 3 * DI], f32, tag="ca_bq", bufs=2, name="ca_bq")
            nc.sync.dma_start(ca_bq[:], ca_bqkvt[l])
            sa_bo_t = sm.tile([128, DI], f32, tag="sa_bo_t", bufs=2, name="sa_bo_t")
            nc.sync.dma_start(sa_bo_t[:], sa_bot[l])
            ca_bo_t = sm.tile([128, DI], f32, tag="ca_bo_t", bufs=2, name="ca_bo_t")
            nc.sync.dma_start(ca_bo_t[:], ca_bot[l])
            f_b1 = sm.tile([128, DFI], f32, tag="f_b1", bufs=2, name="f_b1")
            nc.sync.dma_start(f_b1[:], ff_b1t[l])
            f_b2 = sm.tile([128, DI], f32, tag="f_b2", bufs=2, name="f_b2")
            nc.sync.dma_start(f_b2[:], ff_b2t[l])
            lng = []
            lnb = []
            for k in range(3):
                g = sm.tile([128, DI], f32, tag=f"lng{k}", bufs=2, name=f"lng{k}")
                nc.sync.dma_start(g[:], lngt[l, k])
                lng.append(g)
                b_ = sm.tile([128, DI], f32, tag=f"lnb{k}", bufs=2, name=f"lnb{k}")
                nc.sync.dma_start(b_[:], lnbt[l, k])
                lnb.append(b_)
            bvr = sm.tile([1, D], bf, tag="bvr", bufs=2, name="bvr")
            nc.sync.dma_start(bvr[:], sa_bvrow[l])

            # ======== self-attention ========
            wq = load_w(lambda i: sa_wqkvT[l, i * 128 : (i + 1) * 128, 0:1024])
            wk = load_w(lambda i: sa_wqkvT[l, i * 128 : (i + 1) * 128, 1024:2048])

            vsb = None
            at = [None] * H
            state = {}

            def sa_phase1(h):
                ebd = []
                for kt_i, (k0, kw) in enumerate(KT):
                    e = bb.tile([128, W], bf, tag="eb", bufs=4, name="ebd")
                    nc.sync.dma_start(e[:kw, :], EBD[h, k0 : k0 + kw, :])
                    ebd.append(e)
                qh = bb.tile([128, W], bf, tag="qk", bufs=3, name="qh")
                kh = bb.tile([128, W], bf, tag="qk", bufs=3, name="kh")
                for dst, ws, bcol, scl in ((qh, wq, h, INV), (kh, wk, DI + h, 1.0)):
                    for c0, cw in CH3:
                        p = pk.tile([128, 512], f32, tag="pk", name="pk")
                        for i in range(DI):
                            nc.tensor.matmul(p[:, :cw], ws[i][:, h * 128 : (h + 1) * 128],
                                             hs[i][:, c0 : c0 + cw],
                                             start=(i == 0), stop=(i == DI - 1))
                        nc.scalar.activation(dst[:, c0 : c0 + cw], p[:, :cw], AF.Identity,
                                             bias=sa_bq[:, bcol : bcol + 1], scale=scl)
                pts = []
                for kt_i, (k0, kw) in enumerate(KT):
                    pt = bb.tile([128, W], bf, tag="pts", bufs=10, name="pts")
                    for bq, c0, cw in BQCH:
                        p = pk.tile([128, 512], f32, tag="pk", name="pk")
                        nc.tensor.matmul(p[:kw, :cw],
                                         kh[:, bq * 601 + k0 : bq * 601 + k0 + kw],
                                         qh[:, c0 : c0 + cw], start=True, stop=True)
                        nc.scalar.activation(pt[:kw, c0 : c0 + cw], p[:kw, :cw], AF.Exp)
                    nc.vector.tensor_tensor(pt[:kw, :], pt[:kw, :], ebd[kt_i][:kw, :],
                                            OP.mult)
                    pts.append(pt)
                state[h] = pts

            def sa_phase2(h):
                pts = state.pop(h)
                rr = rw.tile([1, W], f32, tag="rr", bufs=2, name="rr")
                for c0, cw in CH3:
                    dn = pd.tile([1, 512], f32, tag="pd", name="dn")
                    for kt_i, (k0, kw) in enumerate(KT):
                        nc.tensor.matmul(dn[:, :cw], ones_col[:kw, :],
                                         pts[kt_i][:kw, c0 : c0 + cw],
                                         start=(kt_i == 0), stop=(kt_i == 4))
                    nc.vector.reciprocal_approx_fast(rr[:, c0 : c0 + cw], dn[:, :cw])
                a = bb.tile([128, W], bf, tag="acc8", name="at")
                at[h] = a
                avp = [None] * 4

                def av_mm(j):
                    bq, c0, cw = BQCH[j]
                    p = pk.tile([128, 512], f32, tag="pk", name="pk")
                    for kt_i, (k0, kw) in enumerate(KT):
                        nc.tensor.matmul(p[:, :cw],
                                         vsb[bq * 5 + kt_i][:kw, h * 128 : (h + 1) * 128],
                                         pts[kt_i][:kw, c0 : c0 + cw],
                                         start=(kt_i == 0), stop=(kt_i == 4))
                    avp[j] = p

                def av_fin(j):
                    bq, c0, cw = BQCH[j]
                    rbp = pb.tile([128, 512], f32, tag="pb", name="rbp")
                    nc.tensor.matmul(rbp[:, :cw], ones_f[:], rr[:, c0 : c0 + cw],
                                     start=True, stop=True)
                    rbs = rw.tile([128, 301], bf, tag="rbs", bufs=1, name="rbs")
                    nc.scalar.copy(rbs[:, :cw], rbp[:, :cw])
                    nc.vector.tensor_tensor(a[:, c0 : c0 + cw], avp[j][:, :cw],
                                            rbs[:, :cw], OP.mult)

                av_mm(0)
                for j in range(1, 4):
                    av_mm(j)
                    av_fin(j - 1)
                av_fin(3)

            prev = None
            for h in range(H):
                sa_phase1(h)
                if h == 0:
                    # V projection (key-major, both batches), overlaps exp/EB of h0
                    wv = load_w(lambda i: sa_wqkvT[l, i * 128 : (i + 1) * 128, 2048:3072])
                    vsb = []
                    for bq in range(2):
                        for k0, kw in KT:
                            kk = bq * 601 + k0
                            v = bb.tile([128, 1024], bf, tag="vsb", bufs=10, name="vsb")
                            for og in range(2):
                                p = pk.tile([128, 512], f32, tag="pk", name="pk")
                                for i in range(DI):
                                    nc.tensor.matmul(p[:kw, :], hs[i][:, kk : kk + kw],
                                                     wv[i][:, og * 512 : (og + 1) * 512],
                                                     start=(i == 0), stop=False)
                                nc.tensor.matmul(p[:kw, :], ones_b[:, :kw],
                                                 bvr[:, og * 512 : og * 512 + 512],
                                                 start=False, stop=True)
                                nc.scalar.copy(v[:kw, og * 512 : (og + 1) * 512], p[:kw, :])
                            vsb.append(v)
                if prev is not None:
                    sa_phase2(prev)
                prev = h
            sa_phase2(prev)
            out_proj_res(sa_woT[l], sa_bo_t[:], at)
            layer_norm(lng[0][:], lnb[0][:])

            # ======== cross-attention ========
            mx = []
            for i in range(DI):
                m_ = bb.tile([128, W], bf, tag="big", name="mx")
                nc.sync.dma_start(m_[:], memP[i * 128 : (i + 1) * 128, :])
                nc.vector.tensor_copy(m_[:, 0:1], adp[i][:, 0:1])
                nc.vector.tensor_copy(m_[:, A[1] : A[1] + 1], adp[i][:, 1:2])
                mx.append(m_)
            cwq = load_w(lambda i: ca_wqkvT[l, i * 128 : (i + 1) * 128, 0:1024])
            cwk = load_w(lambda i: ca_wqkvT[l, i * 128 : (i + 1) * 128, 1024:2048])
            cwv = load_w(lambda i: ca_wqkvT[l, i * 128 : (i + 1) * 128, 2048:3072])
            for h in range(H):
                qh = bb.tile([128, W], bf, tag="qk", bufs=3, name="cqh")
                kh = bb.tile([128, W], bf, tag="qk", bufs=3, name="ckh")
                vh = bb.tile([128, W], bf, tag="qk", bufs=3, name="cvh")
                for dst, ws, rhs_t, bcol, scl in (
                    (qh, cwq, hs, h, INV),
                    (kh, cwk, mx, DI + h, 1.0),
                    (vh, cwv, mx, 2 * DI + h, 1.0),
                ):
                    for c0, cw in CH3:
                        p = pk.tile([128, 512], f32, tag="pk", name="pk")
                        for i in range(DI):
                            nc.tensor.matmul(p[:, :cw], ws[i][:, h * 128 : (h + 1) * 128],
                                             rhs_t[i][:, c0 : c0 + cw],
                                             start=(i == 0), stop=(i == DI - 1))
                        nc.scalar.activation(dst[:, c0 : c0 + cw], p[:, :cw], AF.Identity,
                                             bias=ca_bq[:, bcol : bcol + 1], scale=scl)
                a = bb.tile([128, W], bf, tag="acc8", name="cat")
                at[h] = a
                wm = rw.tile([1, W], bf, tag="wm", bufs=1, name="wm")
                for bq in range(2):
                    q0 = bq * 601 + 1
                    ka = rw.tile([128, 1], f32, tag="ka", bufs=4, name="ka")
                    nc.vector.tensor_copy(ka[:], kh[:, q0 - 1 : q0])
                    va = rw.tile([128, 1], f32, tag="va", bufs=4, name="va")
                    nc.vector.tensor_copy(va[:], vh[:, q0 - 1 : q0])
                    kd = bb.tile([128, T], bf, tag="kd", bufs=2, name="kd")
                    nc.vector.tensor_scalar_sub(kd[:], kh[:, q0 : q0 + T], ka[:])
                    nc.vector.tensor_tensor(kd[:], kd[:], qh[:, q0 : q0 + T], OP.mult)
                    for sc in range(2):
                        c0, cw = sc * 300, 300
                        dd = pd.tile([1, 512], f32, tag="pd", name="dd")
                        nc.tensor.matmul(dd[:, :cw], ones_col[:], kd[:, c0 : c0 + cw],
                                         start=True, stop=True)
                        nc.scalar.activation(wm[:, q0 + c0 : q0 + c0 + cw], dd[:, :cw],
                                             AF.Sigmoid)
                    vd = bb.tile([128, T], bf, tag="kd", bufs=2, name="vd")
                    nc.vector.tensor_scalar_sub(vd[:], vh[:, q0 : q0 + T], va[:])
                    for sc in range(2):
                        c0, cw = sc * 300, 300
                        wmp = pb.tile([128, 512], f32, tag="pb", name="wmp")
                        nc.tensor.matmul(wmp[:, :cw], ones_b[:],
                                         wm[:, q0 + c0 : q0 + c0 + cw],
                                         start=True, stop=True)
                        tv = bb.tile([128, 401], bf, tag="t1", bufs=2, name="tv")
                        nc.vector.tensor_tensor(tv[:, :cw], vd[:, c0 : c0 + cw],
                                                wmp[:, :cw], OP.mult)
                        nc.scalar.activation(a[:, q0 + c0 : q0 + c0 + cw], tv[:, :cw],
                                             AF.Identity, bias=va[:])
            out_proj_res(ca_woT[l], ca_bo_t[:], at)
            layer_norm(lng[1][:], lnb[1][:])

            # ======== FFN (DFF quarters with bf16 partial accumulation) ========
            part = [None] * DI
            for q4 in range(4):
                w1q = load_w(lambda i: ff_w1T[l, i * 128 : (i + 1) * 128,
                                              q4 * 1024 : (q4 + 1) * 1024])
                ffa = []
                for f in range(8):
                    fa = bb.tile([128, W], bf, tag="big", name="ffa")
                    for c0, cw in CH3:
                        p = pk.tile([128, 512], f32, tag="pk", name="pk")
                        for i in range(DI):
                            nc.tensor.matmul(p[:, :cw], w1q[i][:, f * 128 : (f + 1) * 128],
                                             hs[i][:, c0 : c0 + cw],
                                             start=(i == 0), stop=(i == DI - 1))
                        nc.scalar.activation(fa[:, c0 : c0 + cw], p[:, :cw], AF.Relu,
                                             bias=f_b1[:, q4 * 8 + f : q4 * 8 + f + 1])
                    ffa.append(fa)
                w2q = load_w(lambda ii: ff_w2T[l, (q4 * 8 + ii) * 128 : (q4 * 8 + ii + 1) * 128,
                                               0:1024])
                if q4 == 0:
                    for o in range(DI):
                        part[o] = bb.tile([128, W], bf, tag="acc8", name="part")
                for c0, cw in CH3:
                    for o in range(DI):
                        p = pk.tile([128, 512], f32, tag="pk", name="pk")
                        for ii in range(8):
                            nc.tensor.matmul(p[:, :cw], w2q[ii][:, o * 128 : (o + 1) * 128],
                                             ffa[ii][:, c0 : c0 + cw],
                                             start=(ii == 0), stop=(ii == 7))
                        if q4 == 0:
                            nc.scalar.copy(part[o][:, c0 : c0 + cw], p[:, :cw])
                        elif q4 < 3:
                            nc.vector.tensor_tensor(part[o][:, c0 : c0 + cw],
                                                    part[o][:, c0 : c0 + cw],
                                                    p[:, :cw], OP.add)
                        else:
                            u = bb.tile([128, 401], bf, tag="t1", bufs=2, name="u")
                            nc.vector.tensor_tensor(u[:, :cw], part[o][:, c0 : c0 + cw],
                                                    p[:, :cw], OP.add)
                            nc.vector.scalar_tensor_tensor(hs[o][:, c0 : c0 + cw],
                                                           u[:, :cw], f_b2[:, o : o + 1],
                                                           hs[o][:, c0 : c0 + cw],
                                                           OP.add, OP.add)
            layer_norm(lng[2][:], lnb[2][:])
            refresh_adapters()

        # ---------- output projection ----------
        out_sb = bb.tile([IN, W], bf, tag="eb", bufs=4, name="out_sb")
        for c0, cw in CH3:
            p = pk.tile([128, 512], f32, tag="pk", name="pk")
            for i in range(DI):
                nc.tensor.matmul(p[:IN, :cw], w_out_t[i][:], hs[i][:, c0 : c0 + cw],
                                 start=(i == 0), stop=(i == DI - 1))
            nc.scalar.activation(out_sb[:, c0 : c0 + cw], p[:IN, :cw], AF.Identity,
                                 bias=bo_t[:])
        nc.sync.dma_start(out_d[:], out_sb[:])

        for _pool in (pb, pd, pk, rw, wg, bb, sm, res):
            _pool.release()

    nc.compile()
    return nc


def _prep_host(inputs):
    f32 = np.float32

    def b16(a):
        return np.ascontiguousarray(np.asarray(a, f32)).astype(bf16np)

    def tiled(vec, n):
        return np.ascontiguousarray(np.asarray(vec, f32).reshape(n, 128).T)

    x = np.asarray(inputs["x"], f32)
    memory = np.asarray(inputs["memory"], f32)
    ts = np.asarray(inputs["timesteps"])
    pe = np.asarray(inputs["pe"], f32)
    alibi = np.asarray(inputs["alibi"], f32)

    half = D // 2
    expo = np.exp(-math.log(10000.0) * np.arange(half, dtype=f32) / (half - 1.0))
    efm = np.concatenate([expo, expo]) / (2 * np.pi)
    phs = np.concatenate([np.zeros(half, f32), np.full(half, 0.25, f32)])

    # exp(alibi) with adapter key row, q-cols duplicated for both batches
    eb = np.exp(np.concatenate([np.zeros((H, T, 1), f32), alibi], axis=-1)
                .transpose(0, 2, 1))            # [H, 601, 600]
    EBD = np.ones((H, T + 1, W), f32)
    EBD[:, :, 1 : T + 1] = eb
    EBD[:, :, T + 2 :] = eb

    peP = np.zeros((D, W), f32)
    peb = pe.T + np.asarray(inputs["b_in"], f32)[:, None]
    peP[:, 1 : T + 1] = peb
    peP[:, T + 2 :] = peb

    qkv_bias = {}
    for nm in ("sa", "ca"):
        bq = np.asarray(inputs[f"{nm}_bqkv"], f32).copy()
        bq[:, :D] *= INV
        qkv_bias[nm] = np.stack([np.stack([bq[l, k * 128 : (k + 1) * 128]
                                           for k in range(3 * DI)], axis=1)
                                 for l in range(L)])

    common = {
        "xP": None, "memP": None, "tsf": None,
        "efm": tiled(efm, DI), "phs": tiled(phs, DI),
        "pePB": b16(peP),
        "w_inT": b16(np.asarray(inputs["W_in"], f32).T),
        "te_w1T": b16(np.asarray(inputs["te_W1"], f32).T),
        "te_w2T": b16(np.asarray(inputs["te_W2"], f32).T),
        "te_b1t": tiled(inputs["te_b1"], DI),
        "te_b2t": tiled(inputs["te_b2"], DI),
        "sa_wqkvT": b16(np.asarray(inputs["sa_Wqkv"], f32).transpose(0, 2, 1)),
        "sa_bqkvt": qkv_bias["sa"],
        "sa_bvrow": b16(np.asarray(inputs["sa_bqkv"], f32)[:, 2 * D :][:, None, :]),
        "sa_woT": b16(np.asarray(inputs["sa_Wo"], f32).transpose(0, 2, 1)),
        "sa_bot": np.stack([tiled(np.asarray(inputs["sa_bo"], f32)[l], DI) for l in range(L)]),
        "ca_wqkvT": b16(np.asarray(inputs["ca_Wqkv"], f32).transpose(0, 2, 1)),
        "ca_bqkvt": qkv_bias["ca"],
        "ca_woT": b16(np.asarray(inputs["ca_Wo"], f32).transpose(0, 2, 1)),
        "ca_bot": np.stack([tiled(np.asarray(inputs["ca_bo"], f32)[l], DI) for l in range(L)]),
        "ff_w1T": b16(np.asarray(inputs["ff_W1"], f32).transpose(0, 2, 1)),
        "ff_b1t": np.stack([tiled(np.asarray(inputs["ff_b1"], f32)[l], DFI) for l in range(L)]),
        "ff_w2T": b16(np.asarray(inputs["ff_W2"], f32).transpose(0, 2, 1)),
        "ff_b2t": np.stack([tiled(np.asarray(inputs["ff_b2"], f32)[l], DI) for l in range(L)]),
        "lngt": np.stack([np.stack([tiled(np.asarray(inputs[f"ln{k+1}_g"], f32)[l], DI)
                                    for k in range(3)]) for l in range(L)]),
        "lnbt": np.stack([np.stack([tiled(np.asarray(inputs[f"ln{k+1}_b"], f32)[l], DI)
                                    for k in range(3)]) for l in range(L)]),
        "EBD": EBD.astype(bf16np),
        "w_outT": b16(np.asarray(inputs["W_out"], f32).T),
        "b_out": np.asarray(inputs["b_out"], f32)[:, None],
    }

    in_maps = []
    for c in range(NC):
        b0 = c * BC
        m = dict(common)
        xPc = np.zeros((IN, W), f32)
        xPc[:, 1 : T + 1] = x[b0].T
        xPc[:, T + 2 :] = x[b0 + 1].T
        m["xP"] = xPc.astype(bf16np)
        mPc = np.zeros((D, W), f32)
        mPc[:, 1 : T + 1] = memory[b0].T
        mPc[:, T + 2 :] = memory[b0 + 1].T
        m["memP"] = mPc.astype(bf16np)
        m["tsf"] = np.asarray(ts[b0 : b0 + BC], f32)[None, :]
        in_maps.append(m)
    return in_maps


def kernel(**inputs):
    from concourse.bass_utils import run_bass_kernel_spmd

    if "nc" not in _cache:
        _cache["nc"] = _build()
    nc = _cache["nc"]
    in_maps = _prep_host(inputs)
    res = run_bass_kernel_spmd(nc, in_maps, core_ids=list(range(NC)))
    out = np.empty((B, T, IN), np.float32)
    for c in range(NC):
        o = np.asarray(res.results[c]["out"], dtype=np.float32)
        out[c * BC] = o[:, 1 : T + 1].T
        out[c * BC + 1] = o[:, T + 2 :].T
    return out
